# revision 1
# baseline (speedup 1.0000x reference)
"""Multi-headed causal attention (B=2, S=2048, D=1024, H=16, DK=DV=64) on 8
Trainium2 NeuronCores.

Sharding (zero-communication): cores are split into 2 groups of 4, one group
per batch element. Within a group, core j owns two 256-query stripes: block j
(rows 256j..256j+255) and block 7-j. Stripe A (the early block, j<=3) only
attends to keys [0, 1024); stripe B (block 7-j >= 4) attends to keys
[0, 2048). Each core recomputes the K/V projections for its batch (keys it
needs), computes its queries' attention and output projection rows, and the
host concatenates row slices -- no cross-core communication.

All matmuls run in fp32r (full PE rate at free-dim >= 256, ~1e-4 relative
error). Softmax skips max-subtraction (scores are O(1) by construction, exp
cannot overflow) and gets its denominator from an all-ones column appended to
V, so the whole softmax costs one ACT exp pass plus one DVE mask-multiply.
Causal/validity masking is a 0/1 multiplicative mask input applied post-exp.
Per-head normalization happens on the small [64, 512] attention output (not
the big attention matrix) via a PE-replicated reciprocal row, which lets the
output projection accumulate all 16 heads in PSUM.
"""

import numpy as np

B, S, D, H, DK = 2, 2048, 1024, 16, 64
NQ = 512          # queries per core: 2 stripes x 256
NCORES = 8

_BUILT = {}


def _build_nc():
    import os
    PH = int(os.environ.get("BISECT_PHASES", "6"))
    import concourse.bacc as bacc
    import concourse.mybir as mybir
    from concourse import tile

    f32 = mybir.dt.float32
    f32r = mybir.dt.float32r
    bf16 = mybir.dt.bfloat16
    AF = mybir.ActivationFunctionType
    ALU = mybir.AluOpType

    nc = bacc.Bacc("TRN2", target_bir_lowering=False, debug=False,
                   num_devices=NCORES)

    xk_t = nc.declare_dram_parameter("xk_t", [D, S], f32r, isOutput=False)
    xv_t = nc.declare_dram_parameter("xv_t", [D, S], f32r, isOutput=False)
    xq_t = nc.declare_dram_parameter("xq_t", [D, NQ], f32r, isOutput=False)
    wk_t = nc.declare_dram_parameter("wk_t", [D, D], f32r, isOutput=False)
    wv_t = nc.declare_dram_parameter("wv_t", [D, D], f32r, isOutput=False)
    wq_t = nc.declare_dram_parameter("wq_t", [D, D], f32r, isOutput=False)
    wo_t = nc.declare_dram_parameter("wo_t", [D, D], f32r, isOutput=False)
    bk_s = nc.declare_dram_parameter("bk_s", [128, 8], f32, isOutput=False)
    bq_s = nc.declare_dram_parameter("bq_s", [128, 8], f32, isOutput=False)
    bv_r = nc.declare_dram_parameter("bv_r", [1, D], f32r, isOutput=False)
    bo_r = nc.declare_dram_parameter("bo_r", [1, D], f32r, isOutput=False)
    ones1 = nc.declare_dram_parameter("ones1", [1, 128], f32r, isOutput=False)
    ones128 = nc.declare_dram_parameter("ones128", [128, 128], f32r, isOutput=False)
    onesv = nc.declare_dram_parameter("onesv", [128, 8], f32r, isOutput=False)
    maskin = nc.declare_dram_parameter("maskin", [S, 2 * NQ], bf16, isOutput=False)
    out = nc.declare_dram_parameter("out", [NQ, D], f32, isOutput=True)

    from contextlib import ExitStack

    class _Stop(Exception):
        pass

    with tile.TileContext(nc) as tc:
      try:
        with ExitStack() as ctx:
            persist = ctx.enter_context(tc.tile_pool(name="persist", bufs=1))
            w2 = ctx.enter_context(tc.tile_pool(name="w2", bufs=2))
            w3 = ctx.enter_context(tc.tile_pool(name="w3", bufs=3))

            # ---- constants ----
            bk_sb = persist.tile([128, 8], f32, name="bk", tag="bk")
            bq_sb = persist.tile([128, 8], f32, name="bq", tag="bq")
            ones_sb = persist.tile([1, 128], f32r, name="ones1", tag="ones1")
            nc.sync.dma_start(bk_sb[:], bk_s[:])
            nc.sync.dma_start(bq_sb[:], bq_s[:])
            nc.sync.dma_start(ones_sb[:], ones1[:])
            ones128_sb = persist.tile([128, 128], f32r, name="ones128",
                                      tag="ones128")
            nc.sync.dma_start(ones128_sb[:], ones128[:])
            # ---- P1: replicate bv, bo across partitions via K=1 matmul ----
            bv_rep = persist.tile([128, D], f32, name="bvrep", tag="bvrep")
            with tc.tile_pool(name="ps1", bufs=2, space="PSUM") as ps1, \
                 tc.tile_pool(name="p1s", bufs=1) as p1s:
                bv_rsb = p1s.tile([1, D], f32r, name="bvr", tag="bvr")
                nc.sync.dma_start(bv_rsb[:], bv_r[:])
                for half in range(2):
                    rp = ps1.tile([128, 512], f32, name="rep1", tag="rep1")
                    nc.tensor.matmul(rp[:], ones_sb[:],
                                     bv_rsb[:, half * 512:(half + 1) * 512],
                                     start=True, stop=True)
                    nc.scalar.copy(bv_rep[:, half * 512:(half + 1) * 512],
                                   rp[:])

            # ---- P2: kT projection: kT[ft] = (Wk x_k^T + bk)[ft] ----
            phase_ctx = ctx.enter_context(ExitStack())
            projp = phase_ctx.enter_context(tc.tile_pool(name="projp", bufs=1))
            kT = [projp.tile([128, S], f32r, name=f"kt{ft}", tag=f"kt{ft}")
                  for ft in range(8)]
            with tc.tile_pool(name="wkp", bufs=1) as wkp, \
                 tc.tile_pool(name="ps2", bufs=3, space="PSUM") as ps2:
                wk_sb = [wkp.tile([128, D], f32r, name=f"wk{kp}", tag=f"wk{kp}")
                         for kp in range(8)]
                for kp in range(8):
                    nc.sync.dma_start(wk_sb[kp][:],
                                      wk_t[kp * 128:(kp + 1) * 128, :])
                for sc in range(4):
                    xkc = [w2.tile([128, 512], f32r, name=f"x{kp}", tag=f"x{kp}")
                           for kp in range(8)]
                    for kp in range(8):
                        nc.sync.dma_start(
                            xkc[kp][:],
                            xk_t[kp * 128:(kp + 1) * 128,
                                 sc * 512:(sc + 1) * 512])
                    for ft in range(8):
                        ps = ps2.tile([128, 512], f32, name="p2", tag="p2")
                        for kp in range(8):
                            nc.tensor.matmul(
                                ps[:],
                                wk_sb[kp][:, ft * 128:(ft + 1) * 128],
                                xkc[kp][:],
                                start=(kp == 0), stop=(kp == 7))
                        nc.scalar.activation(
                            kT[ft][:, sc * 512:(sc + 1) * 512], ps[:],
                            AF.Identity, bias=bk_sb[:, ft:ft + 1])

            # ---- P3: qT projection ----
            if PH < 3:
                raise _Stop()
            qT = [projp.tile([128, NQ], f32r, name=f"qt{ft}", tag=f"qt{ft}")
                  for ft in range(8)]
            with tc.tile_pool(name="wqp", bufs=1) as wqp, \
                 tc.tile_pool(name="ps3", bufs=3, space="PSUM") as ps3:
                wq_sb = [wqp.tile([128, D], f32r, name=f"wq{kp}", tag=f"wq{kp}")
                         for kp in range(8)]
                xqc = [w2.tile([128, NQ], f32r, name=f"x{kp}", tag=f"x{kp}")
                       for kp in range(8)]
                for kp in range(8):
                    nc.sync.dma_start(wq_sb[kp][:],
                                      wq_t[kp * 128:(kp + 1) * 128, :])
                    nc.sync.dma_start(xqc[kp][:],
                                      xq_t[kp * 128:(kp + 1) * 128, :])
                for ft in range(8):
                    ps = ps3.tile([128, NQ], f32, name="p3", tag="p3")
                    for kp in range(8):
                        nc.tensor.matmul(
                            ps[:], wq_sb[kp][:, ft * 128:(ft + 1) * 128],
                            xqc[kp][:], start=(kp == 0), stop=(kp == 7))
                    nc.scalar.activation(qT[ft][:], ps[:], AF.Identity,
                                         bias=bq_sb[:, ft:ft + 1])

            # ---- P4: masks ----
            if PH < 4:
                raise _Stop()
            attnp = phase_ctx.enter_context(tc.tile_pool(name="attnp", bufs=1))
            mask_sb = [attnp.tile([128, 2 * NQ], bf16, name=f"mk{p}",
                                  tag=f"mk{p}")
                       for p in range(8)]
            for p in range(8):
                nc.sync.dma_start(mask_sb[p][:],
                                  maskin[p * 128:(p + 1) * 128, :])

            # ---- P5: per 4-head group: V projection + paired attention ----
            if PH < 5:
                raise _Stop()
            navTn = [persist.tile([128, NQ], f32r, name=f"nv{i}", tag=f"nv{i}")
                     for i in range(8)]
            with tc.tile_pool(name="p5", bufs=1) as p5, \
                 tc.tile_pool(name="p5n", bufs=2) as p5n, \
                 tc.tile_pool(name="p5c", bufs=3, space="PSUM") as p5sc, \
                 tc.tile_pool(name="p5v", bufs=2, space="PSUM") as p5vp, \
                 tc.tile_pool(name="p5a0", bufs=1, space="PSUM") as p5a0, \
                 tc.tile_pool(name="p5a1", bufs=1, space="PSUM") as p5a1, \
                 tc.tile_pool(name="p5r", bufs=1, space="PSUM") as p5rp:
                for hg in range(4):
                    # V projection for heads 4hg..4hg+3
                    wv_sb = [p5.tile([128, 256], f32r, name=f"wv{kp}",
                                     tag=f"wv{kp}")
                             for kp in range(8)]
                    for kp in range(8):
                        nc.sync.dma_start(
                            wv_sb[kp][:],
                            wv_t[kp * 128:(kp + 1) * 128,
                                 hg * 256:(hg + 1) * 256])
                    v_hg = [attnp.tile([128, 260], f32r, name=f"v{st}",
                                       tag=f"v{st}")
                            for st in range(16)]
                    for st in range(16):
                        nc.sync.dma_start(
                            v_hg[st][:].rearrange("p (h c) -> p h c",
                                                  c=65)[:, :, 64:65],
                            onesv[:, 0:4].rearrange("p (h c) -> p h c", c=1))
                    for chunk in range(4):
                        xvc = [w2.tile([128, 512], f32r, name=f"x{kp}",
                                       tag=f"x{kp}")
                               for kp in range(8)]
                        for kp in range(8):
                            nc.sync.dma_start(
                                xvc[kp][:],
                                xv_t[kp * 128:(kp + 1) * 128,
                                     chunk * 512:(chunk + 1) * 512])
                        for stl in range(4):
                            st = 4 * chunk + stl
                            vp = p5vp.tile([128, 256], f32, name="vp",
                                           tag="vp")
                            for kp in range(8):
                                nc.tensor.matmul(
                                    vp[:],
                                    xvc[kp][:, stl * 128:(stl + 1) * 128],
                                    wv_sb[kp][:],
                                    start=(kp == 0), stop=(kp == 7))
                            nc.vector.tensor_tensor(
                                v_hg[st][:].rearrange(
                                    "p (h c) -> p h c", c=65)[:, :, 0:64],
                                vp[:].rearrange("p (h c) -> p h c", c=64),
                                bv_rep[:, hg * 256:(hg + 1) * 256].rearrange(
                                    "p (h c) -> p h c", c=64),
                                ALU.add)
                    # attention: 2 head-pairs, kt-interleaved so the two
                    # heads' K=64 scores matmuls land in disjoint PE row
                    # groups and run concurrently
                    for pl in range(2):
                        hp = 2 * hg + pl
                        dgat = p5n.tile([128, NQ], f32, name="dgat",
                                        tag="dgat")
                        nc.gpsimd.memset(dgat[:], 1.0)
                        avp = [p5a0.tile([65, NQ], f32, name="av0",
                                         tag="av0"),
                               p5a1.tile([65, NQ], f32, name="av1",
                                         tag="av1")]
                        for i, hs in ((0, 0), (1, 64)):
                            for p in range(8):
                                kt0 = 2 * p
                                N, qoff = (512, 0) if kt0 < 8 else (256, 256)
                                am = w3.tile([128, 2 * NQ], f32r, name="am",
                                             tag="am")
                                for half in range(2):
                                    kt = kt0 + half
                                    sc_ps = p5sc.tile([128, 512], f32,
                                                      name="sc", tag="sc")
                                    nc.tensor.matmul(
                                        sc_ps[:, 0:N],
                                        kT[hp][hs:hs + 64,
                                               kt * 128:(kt + 1) * 128],
                                        qT[hp][hs:hs + 64, qoff:512],
                                        start=True, stop=True)
                                    nc.scalar.activation(
                                        am[:, half * NQ + qoff:
                                           half * NQ + qoff + N],
                                        sc_ps[:, 0:N], AF.Exp, scale=0.125)
                                nc.vector.tensor_tensor(
                                    am[:].rearrange(
                                        "x (h q) -> x h q",
                                        q=NQ)[:, :, qoff:qoff + N],
                                    am[:].rearrange(
                                        "x (h q) -> x h q",
                                        q=NQ)[:, :, qoff:qoff + N],
                                    mask_sb[p][:].rearrange(
                                        "x (h q) -> x h q",
                                        q=NQ)[:, :, qoff:qoff + N],
                                    ALU.mult)
                                for half in range(2):
                                    kt = kt0 + half
                                    nc.tensor.matmul(
                                        avp[i][:, qoff:qoff + N],
                                        v_hg[kt][:, (2 * pl + i) * 65:
                                                 (2 * pl + i + 1) * 65],
                                        am[:, half * NQ + qoff:
                                           half * NQ + qoff + N],
                                        start=(kt == 0), stop=(kt == 15))
                        for i in range(2):
                            nc.scalar.copy(dgat[64 * i:64 * i + 1, :],
                                           avp[i][64:65, :])
                            nc.scalar.copy(navTn[hp][64 * i:64 * i + 64, :],
                                           avp[i][0:64, :])
                        dgrec = p5n.tile([128, NQ], f32r, name="dgrec",
                                         tag="dgrec")
                        with nc.allow_low_precision(
                                reason="f32r recip, ~5e-4 rel ok"):
                            nc.vector.reciprocal(dgrec[:], dgat[:])
                        for i in range(2):
                            rep_ps = p5rp.tile([64, NQ], f32, name="repd",
                                               tag="repd")
                            nc.tensor.matmul(
                                rep_ps[:],
                                ones128_sb[64 * i:64 * i + 1, 0:64],
                                dgrec[64 * i:64 * i + 1, :],
                                start=True, stop=True)
                            nc.vector.tensor_tensor(
                                navTn[hp][64 * i:64 * i + 64, :],
                                navTn[hp][64 * i:64 * i + 64, :],
                                rep_ps[:], ALU.mult)

            phase_ctx.close()

            # ---- P6: output projection, all heads PSUM-accumulated ----
            if PH < 6:
                raise _Stop()
            with tc.tile_pool(name="p6", bufs=2) as p6, \
                 tc.tile_pool(name="ps6", bufs=2, space="PSUM") as ps6:
                bo_rsb = p6.tile([1, D], f32r, name="bor", tag="bor")
                nc.sync.dma_start(bo_rsb[:], bo_r[:])
                bo_rep = p6.tile([128, D], f32, name="borep", tag="borep")
                for half in range(2):
                    rp6 = ps6.tile([128, 512], f32, name="fin", tag="fin")
                    nc.tensor.matmul(rp6[:], ones_sb[:],
                                     bo_rsb[:, half * 512:(half + 1) * 512],
                                     start=True, stop=True)
                    nc.scalar.copy(bo_rep[:, half * 512:(half + 1) * 512],
                                   rp6[:])
                for oc in range(2):
                    wo_sb = [p6.tile([128, 512], f32r, name=f"wo{i}", tag=f"wo{i}")
                             for i in range(8)]
                    for i in range(8):
                        nc.sync.dma_start(
                            wo_sb[i][0:64, :],
                            wo_t[(2 * i) * 64:(2 * i + 1) * 64,
                                 oc * 512:(oc + 1) * 512])
                        nc.sync.dma_start(
                            wo_sb[i][64:128, :],
                            wo_t[(2 * i + 1) * 64:(2 * i + 2) * 64,
                                 oc * 512:(oc + 1) * 512])
                    for rc in range(4):
                        fp = ps6.tile([128, 512], f32, name="fin", tag="fin")
                        for hp in range(8):
                            nc.tensor.matmul(
                                fp[:],
                                navTn[hp][:, rc * 128:(rc + 1) * 128],
                                wo_sb[hp][:],
                                start=(hp == 0), stop=(hp == 7))
                        fo = p6.tile([128, 512], f32, name="fo", tag="fo")
                        nc.vector.tensor_tensor(
                            fo[:], fp[:],
                            bo_rep[:, oc * 512:(oc + 1) * 512], ALU.add)
                        nc.sync.dma_start(
                            out[rc * 128:(rc + 1) * 128,
                                oc * 512:(oc + 1) * 512], fo[:])
      except _Stop:
          pass
    nc.compile()
    return nc


def kernel(V, K, Q, padding_mask, Wv_w, Wv_b, Wk_w, Wk_b, Wq_w, Wq_b,
           Wo_w, Wo_b):
    from concourse.bass_utils import run_bass_kernel_spmd

    V = np.asarray(V, np.float32)
    K = np.asarray(K, np.float32)
    Q = np.asarray(Q, np.float32)
    padding_mask = np.asarray(padding_mask)
    import ml_dtypes

    if "nc" not in _BUILT:
        _BUILT["nc"] = _build_nc()
    nc = _BUILT["nc"]

    wk_t = np.ascontiguousarray(np.asarray(Wk_w, np.float32).T)
    wv_t = np.ascontiguousarray(np.asarray(Wv_w, np.float32).T)
    wq_t = np.ascontiguousarray(np.asarray(Wq_w, np.float32).T)
    wo_t = np.ascontiguousarray(np.asarray(Wo_w, np.float32).T)
    bk_s = np.ascontiguousarray(np.asarray(Wk_b, np.float32).reshape(8, 128).T)
    bq_s = np.ascontiguousarray(np.asarray(Wq_b, np.float32).reshape(8, 128).T)
    bv_r = np.asarray(Wv_b, np.float32).reshape(1, D)
    bo_r = np.asarray(Wo_b, np.float32).reshape(1, D)
    ones1 = np.ones((1, 128), np.float32)
    ones128a = np.ones((128, 128), np.float32)
    onesv = np.ones((128, 8), np.float32)

    xk_T = [np.ascontiguousarray(K[b].T) for b in range(B)]
    xv_T = [np.ascontiguousarray(V[b].T) for b in range(B)]

    in_maps = []
    blocks = []
    kpos = np.arange(S)[:, None]
    for core in range(NCORES):
        b, j = core // 4, core % 4
        blkA, blkB = j, 7 - j
        blocks.append((b, blkA, blkB))
        rows = np.r_[256 * blkA:256 * (blkA + 1), 256 * blkB:256 * (blkB + 1)]
        xq_t = np.ascontiguousarray(Q[b][rows].T)
        qpos = np.r_[np.arange(256 * blkA, 256 * (blkA + 1)),
                     np.arange(256 * blkB, 256 * (blkB + 1))][None, :]
        mask = (kpos <= qpos) & (padding_mask[b][:, None] != 0)
        mp = mask.reshape(16, 128, NQ)
        mask = np.concatenate([mp[0::2], mp[1::2]], axis=2).reshape(S // 2,
                                                                    2 * NQ)
        mask = np.concatenate([mask, np.zeros_like(mask)], axis=0)
        in_maps.append({
            "xk_t": xk_T[b], "xv_t": xv_T[b], "xq_t": xq_t,
            "wk_t": wk_t, "wv_t": wv_t, "wq_t": wq_t, "wo_t": wo_t,
            "bk_s": bk_s, "bq_s": bq_s, "bv_r": bv_r, "bo_r": bo_r,
            "ones1": ones1, "ones128": ones128a, "onesv": onesv,
            "maskin": mask.astype(ml_dtypes.bfloat16),
        })

    _BUILT["last_maps"] = in_maps
    res = run_bass_kernel_spmd(nc, in_maps, core_ids=list(range(NCORES)))
    _BUILT["last_result"] = res

    outf = np.empty((B, S, D), np.float32)
    for core in range(NCORES):
        b, blkA, blkB = blocks[core]
        o = res.results[core]["out"]
        outf[b, 256 * blkA:256 * (blkA + 1)] = o[0:256]
        outf[b, 256 * blkB:256 * (blkB + 1)] = o[256:512]
    return outf



# revision 20
# speedup vs baseline: 1.2821x; 1.2821x over previous
"""Multi-headed causal attention (B=2, S=2048, D=1024, H=16, DK=DV=64) on 8
Trainium2 NeuronCores.

Sharding (zero-communication): cores split into 2 groups of 4, one group per
batch element. Within a group, core g owns four 128-query stripes, one per
"slot" s=0..3 with a key-block budget of 4(s+1) 128-key blocks. Stripe
assignment per group is chosen so every stripe's causal key-need fits its
slot budget:
    g0: stripes [0, 7, 8, 15], g1: [1, 6, 9, 14],
    g2: [2, 5, 10, 13],        g3: [3, 4, 11, 12]
The program is identical on all cores (SPMD); per-core variation lives only
in the data (query permutation in xq_t, 0/1 mask tiles, output row
unpermutation on host).

All matmul operands are bf16 (fast weight load, full PE rate at any free
size); accumulation is fp32 in PSUM. Scores for key-block kb are computed
only for query slots s >= kb//4 ("staircase"): lower slots never attend
those keys. exp(0.125*x) runs on the ACT engine per PSUM bank into bf16 am
tiles; masking is a multiplicative 0/1 bf16 op on am and touches only the
staircase edge (within quad q = kb//4, only slot-q's 128 columns can be
partial or dead). The softmax denominator comes from an all-ones 65th
column appended to each V tile; per-head normalization uses a fast DVE
reciprocal + PE row-replication on the small [64, 512] attention output,
letting the output projection accumulate all 16 heads in PSUM. Elementwise
engines only ever touch PSUM through single-bank [128, 512] fp32 tiles
(hardware constraint); projection bias-adds run on DVE/GpSimd
(tensor_scalar_add) to keep ACT free for exp.
"""

import numpy as np

B, S, D, H, DK = 2, 2048, 1024, 16, 64
NQ = 512          # queries per core: 4 slots x 128
NCORES = 8

SLOT_STRIPES = [
    [0, 7, 8, 15],
    [1, 6, 9, 14],
    [2, 5, 10, 13],
    [3, 4, 11, 12],
]

_BUILT = {}


def _build_nc():
    import os
    PH = int(os.environ.get("BISECT_PHASES", "9"))
    import concourse.bacc as bacc
    import concourse.mybir as mybir
    from concourse import tile

    f32 = mybir.dt.float32
    f32r = mybir.dt.float32r
    bf16 = mybir.dt.bfloat16
    AF = mybir.ActivationFunctionType
    ALU = mybir.AluOpType

    nc = bacc.Bacc("TRN2", target_bir_lowering=False, debug=False,
                   num_devices=NCORES)

    xk_t = nc.declare_dram_parameter("xk_t", [D, S], bf16, isOutput=False)
    xv_t = nc.declare_dram_parameter("xv_t", [D, S], bf16, isOutput=False)
    xq_t = nc.declare_dram_parameter("xq_t", [D, NQ], bf16, isOutput=False)
    wk_t = nc.declare_dram_parameter("wk_t", [D, D], bf16, isOutput=False)
    wv_t = nc.declare_dram_parameter("wv_t", [D, D], bf16, isOutput=False)
    wq_t = nc.declare_dram_parameter("wq_t", [D, D], bf16, isOutput=False)
    wo_t = nc.declare_dram_parameter("wo_t", [D, D], bf16, isOutput=False)
    bk_s = nc.declare_dram_parameter("bk_s", [128, 8], f32, isOutput=False)
    bq_s = nc.declare_dram_parameter("bq_s", [128, 8], f32, isOutput=False)
    bv_rep_d = nc.declare_dram_parameter("bv_rep", [128, D], f32,
                                         isOutput=False)
    bo_rep_d = nc.declare_dram_parameter("bo_rep", [128, D], f32,
                                         isOutput=False)
    onesf = nc.declare_dram_parameter("onesf", [128, 128], f32r,
                                  isOutput=False)
    maskin = nc.declare_dram_parameter("maskin", [128, 4 * NQ], bf16,
                                       isOutput=False)
    out = nc.declare_dram_parameter("out", [NQ, D], f32, isOutput=True)

    from contextlib import ExitStack

    class _Stop(Exception):
        pass

    with tile.TileContext(nc) as tc:
      try:
        with ExitStack() as ctx:
            persist = ctx.enter_context(tc.tile_pool(name="persist", bufs=1))

            # ---- constants / persistent tiles ----
            bk_sb = persist.tile([128, 8], f32, name="bk", tag="bk")
            bq_sb = persist.tile([128, 8], f32, name="bq", tag="bq")
            onesf_sb = persist.tile([128, 128], f32r, name="onesf", tag="onesf")
            mask_sb = persist.tile([128, 4 * NQ], bf16, name="mask",
                                   tag="mask")
            bv_rep = persist.tile([128, D], f32, name="bvrep", tag="bvrep")
            bo_rep = persist.tile([128, D], f32, name="borep", tag="borep")
            kT = [persist.tile([128, S], bf16, name=f"kt{ft}", tag=f"kt{ft}")
                  for ft in range(8)]
            qT = [persist.tile([128, NQ], bf16, name=f"qt{ft}", tag=f"qt{ft}")
                  for ft in range(8)]
            v = [persist.tile([128, 66 * H], bf16, name=f"v{kb}",
                              tag=f"v{kb}")
                 for kb in range(16)]
            navTn = [persist.tile([128, NQ], bf16, name=f"nv{i}",
                                  tag=f"nv{i}")
                     for i in range(8)]

            # p1pool (kT inputs) opened before p2pool so pool release is
            # LIFO (p2 closes first); DMA issue order below still puts the
            # q-phase inputs first.
            p1pool = tc.tile_pool(name="p1pool", bufs=1)
            p1s = p1pool.__enter__()
            p2pool = tc.tile_pool(name="p2pool", bufs=1)
            p2s = p2pool.__enter__()
            wq_sb = [p2s.tile([128, D], bf16, name=f"wq{kp}", tag=f"wq{kp}")
                     for kp in range(8)]
            xq_sb = [p2s.tile([128, NQ], bf16, name=f"xq{kp}", tag=f"xq{kp}")
                     for kp in range(8)]
            for kp in range(8):
                nc.sync.dma_start(wq_sb[kp][:],
                                  wq_t[kp * 128:(kp + 1) * 128, :])
                nc.sync.dma_start(xq_sb[kp][:],
                                  xq_t[kp * 128:(kp + 1) * 128, :])
            nc.sync.dma_start(bq_sb[:], bq_s[:])
            nc.sync.dma_start(bk_sb[:], bk_s[:])
            nc.sync.dma_start(onesf_sb[:], onesf[:])
            nc.sync.dma_start(mask_sb[:], maskin[:])
            nc.sync.dma_start(bv_rep[:], bv_rep_d[:])
            nc.sync.dma_start(bo_rep[:], bo_rep_d[:])

            wk_sb = [p1s.tile([128, D], bf16, name=f"wk{kp}", tag=f"wk{kp}")
                     for kp in range(8)]
            xk_sb = [p1s.tile([128, S], bf16, name=f"xk{kp}", tag=f"xk{kp}")
                     for kp in range(8)]
            for kp in range(8):
                nc.sync.dma_start(wk_sb[kp][:],
                                  wk_t[kp * 128:(kp + 1) * 128, :])
            for kp in range(8):
                nc.sync.dma_start(xk_sb[kp][:],
                                  xk_t[kp * 128:(kp + 1) * 128, :])

            # ---- P2: qT projection (first: smallest DMA footprint) ----
            if PH < 2:
                raise _Stop()
            with tc.tile_pool(name="psq", bufs=2, space="PSUM") as psq:
                for ft in range(8):
                    ps = psq.tile([128, NQ], f32, name="pq", tag="pq")
                    for kp in range(8):
                        nc.tensor.matmul(
                            ps[:], wq_sb[kp][:, ft * 128:(ft + 1) * 128],
                            xq_sb[kp][:], start=(kp == 0), stop=(kp == 7))
                    nc.vector.tensor_scalar_add(qT[ft][:], ps[:],
                                                bq_sb[:, ft:ft + 1])
            p2pool.__exit__(None, None, None)

            # ---- P1: kT projection ----
            if PH < 3:
                raise _Stop()
            with tc.tile_pool(name="psk", bufs=2, space="PSUM") as psk:
                for ft in range(8):
                    ps4 = [psk.tile([128, 512], f32, name=f"pk{sc}",
                                    tag=f"pk{sc}")
                           for sc in range(4)]
                    for kp in range(8):
                        for sc in range(4):
                            nc.tensor.matmul(
                                ps4[sc][:],
                                wk_sb[kp][:, ft * 128:(ft + 1) * 128],
                                xk_sb[kp][:, 512 * sc:512 * (sc + 1)],
                                start=(kp == 0), stop=(kp == 7))
                    for sc in range(4):
                        nc.vector.tensor_scalar_add(
                            kT[ft][:, 512 * sc:512 * (sc + 1)],
                            ps4[sc][:], bk_sb[:, ft:ft + 1])
            p1pool.__exit__(None, None, None)

            # ---- P3: V projection (single pass over xv, all 16 heads) ----
            if PH < 4:
                raise _Stop()
            # ones column (col 64 of each head's 66-wide group) for the
            # softmax denominator; pad col 65 is don't-care.
            for kb in range(16):
                nc.gpsimd.memset(
                    v[kb][:].rearrange("p (h c) -> p h c", c=66)[:, :, 64:65],
                    1.0)
            with tc.tile_pool(name="wvp", bufs=1) as wvp, \
                 tc.tile_pool(name="xvp", bufs=2) as xvp, \
                 tc.tile_pool(name="psv", bufs=3, space="PSUM") as psv:
                wv_sb = [wvp.tile([128, D], bf16, name=f"wv{kp}",
                                  tag=f"wv{kp}")
                         for kp in range(8)]
                for kp in range(8):
                    nc.sync.dma_start(wv_sb[kp][:],
                                      wv_t[kp * 128:(kp + 1) * 128, :])
                for chunk in range(4):
                    xvc = [xvp.tile([128, 512], bf16, name=f"xv{kp}",
                                    tag=f"xv{kp}")
                           for kp in range(8)]
                    for kp in range(8):
                        nc.sync.dma_start(
                            xvc[kp][:],
                            xv_t[kp * 128:(kp + 1) * 128,
                                 chunk * 512:(chunk + 1) * 512])
                    for stl in range(4):
                        kb = 4 * chunk + stl
                        pv = [psv.tile([128, 512], f32, name=f"pv{half}",
                                       tag=f"pv{half}")
                              for half in range(2)]
                        for kp in range(8):
                            for half in range(2):
                                nc.tensor.matmul(
                                    pv[half][:],
                                    xvc[kp][:, stl * 128:(stl + 1) * 128],
                                    wv_sb[kp][:, 512 * half:512 * (half + 1)],
                                    start=(kp == 0), stop=(kp == 7))
                        for half in range(2):
                            nc.vector.tensor_tensor(
                                v[kb][:, 528 * half:528 * (half + 1)]
                                .rearrange("p (h c) -> p h c", c=66)[
                                    :, :, 0:64],
                                pv[half][:].rearrange("p (h c) -> p h c",
                                                      c=64),
                                bv_rep[:, 512 * half:512 * (half + 1)]
                                .rearrange("p (h c) -> p h c", c=64),
                                ALU.add)

            # ---- P4: attention, software-pipelined over (head, kb-pair) ----
            if PH < 5:
                raise _Stop()
            with tc.tile_pool(name="scp", bufs=3, space="PSUM") as scp, \
                 tc.tile_pool(name="avp", bufs=3, space="PSUM") as avpp, \
                 tc.tile_pool(name="repp", bufs=2, space="PSUM") as repp, \
                 tc.tile_pool(name="dnmp", bufs=2) as dnmp, \
                 tc.tile_pool(name="amp", bufs=6) as amp:
                units = [(h, pr) for h in range(H) for pr in range(8)]
                avt = {}    # h -> avp psum tile
                amt = {}    # (h, pr) -> am tile
                dnms = {}   # h -> denominator sbuf tile
                deferred = []   # list of (due_idx, fn)

                def do_scores(h, pr):
                    hp, hs = h // 2, 64 * (h % 2)
                    q = pr // 2
                    Nq = 512 - 128 * q
                    qoff = 128 * q
                    am = amp.tile([128, 1024], bf16, name="am", tag="am")
                    for u in range(2):
                        kb = 2 * pr + u
                        sc = scp.tile([128, 512], f32, name="sc", tag="sc")
                        nc.tensor.matmul(
                            sc[:, 0:Nq],
                            kT[hp][hs:hs + 64, kb * 128:(kb + 1) * 128],
                            qT[hp][hs:hs + 64, qoff:512],
                            start=True, stop=True)
                        nc.scalar.activation(
                            am[:, 512 * u:512 * u + Nq], sc[:, 0:Nq],
                            AF.Exp, scale=0.125)
                    # multiplicative 0/1 mask on the staircase edge: only
                    # slot-q's 128 columns of each block can be partial/dead
                    amv = am[:].rearrange("p (u c) -> p u c",
                                          c=512)[:, :, 0:128]
                    moff = 256 * pr
                    mkv = mask_sb[:, moff:moff + 256].rearrange(
                        "p (u c) -> p u c", c=128)
                    eng = nc.vector if h % 2 == 0 else nc.gpsimd
                    eng.tensor_tensor(amv, amv, mkv, ALU.mult)
                    amt[(h, pr)] = am

                def do_av(h, pr):
                    q = pr // 2
                    Nq = 512 - 128 * q
                    qoff = 128 * q
                    am = amt.pop((h, pr))
                    for u in range(2):
                        kb = 2 * pr + u
                        nc.tensor.matmul(
                            avt[h][0:65, qoff:qoff + Nq],
                            v[kb][:, 66 * h:66 * h + 65],
                            am[:, 512 * u:512 * u + Nq],
                            start=(kb == 0), stop=(kb == 15))

                def do_norm_copy(h):
                    # denoms of a head pair: PSUM partition 64 -> SBUF rows
                    # 0 (even head) and 64 (odd head)
                    hp = h // 2
                    if h % 2 == 0:
                        dnm = dnmp.tile([128, NQ], f32, name="dnm",
                                        tag="dnm")
                        dnms[hp] = dnm
                    nc.scalar.copy(dnms[hp][64 * (h % 2):64 * (h % 2) + 1, :],
                                   avt[h][64:65, :])

                def make_norm_rep(hp):
                    def fn():
                        rcp = dnmp.tile([128, NQ], f32r, name="rcp",
                                        tag="rcp")
                        # rows other than 0/64 are junk and never read
                        with nc.allow_low_precision(reason="f32r recip"):
                            nc.vector.reciprocal(rcp[:], dnms.pop(hp)[:])
                        for i in range(2):
                            rep = repp.tile([128, NQ], f32, name="rep",
                                            tag="rep")
                            nc.tensor.matmul(
                                rep[0:64, :],
                                onesf_sb[64 * i:64 * i + 1, 0:64],
                                rcp[64 * i:64 * i + 1, :],
                                start=True, stop=True)
                            repS = dnmp.tile([128, NQ], f32, name="repS",
                                             tag="repS")
                            nc.vector.tensor_copy(repS[0:64, :],
                                                  rep[0:64, :])
                            with nc.allow_low_precision(
                                    reason="softmax norm"):
                                nc.vector.tensor_tensor(
                                    navTn[hp][64 * i:64 * i + 64, :],
                                    avt.pop(2 * hp + i)[0:64, :],
                                    repS[0:64, :], ALU.mult)
                    return fn

                for i, (h, pr) in enumerate(units):
                    if pr == 0:
                        avt[h] = avpp.tile([128, NQ], f32, name="av",
                                           tag="av")
                    do_scores(h, pr)
                    # run deferred items that are due
                    still = []
                    for due, fn in deferred:
                        if i >= due:
                            fn()
                        else:
                            still.append((due, fn))
                    deferred = still
                    if i >= 1:
                        ph, ppr = units[i - 1]
                        do_av(ph, ppr)
                        if ppr == 7:
                            do_norm_copy(ph)
                            if ph % 2 == 1:
                                deferred.append((i + 1,
                                                 make_norm_rep(ph // 2)))
                # drain
                ph, ppr = units[-1]
                do_av(ph, ppr)
                do_norm_copy(ph)
                for _, fn in deferred:
                    fn()
                make_norm_rep(ph // 2)()

            # ---- P5: output projection, all heads PSUM-accumulated ----
            if PH < 6:
                raise _Stop()
            with tc.tile_pool(name="wop", bufs=1) as wop, \
                 tc.tile_pool(name="fop", bufs=2) as fop, \
                 tc.tile_pool(name="pso", bufs=2, space="PSUM") as pso:
                wo_sb = [wop.tile([128, D], bf16, name=f"wo{hp}",
                                  tag=f"wo{hp}")
                         for hp in range(8)]
                for hp in range(8):
                    nc.sync.dma_start(wo_sb[hp][:],
                                      wo_t[hp * 128:(hp + 1) * 128, :])
                for rc in range(4):
                    po = [pso.tile([128, 512], f32, name=f"po{oc}",
                                   tag=f"po{oc}")
                          for oc in range(2)]
                    for hp in range(8):
                        for oc in range(2):
                            nc.tensor.matmul(
                                po[oc][:],
                                navTn[hp][:, rc * 128:(rc + 1) * 128],
                                wo_sb[hp][:, 512 * oc:512 * (oc + 1)],
                                start=(hp == 0), stop=(hp == 7))
                    fo = fop.tile([128, 1024], f32, name="fo", tag="fo")
                    for oc in range(2):
                        nc.vector.tensor_tensor(
                            fo[:, 512 * oc:512 * (oc + 1)],
                            po[oc][:],
                            bo_rep[:, 512 * oc:512 * (oc + 1)], ALU.add)
                    nc.sync.dma_start(out[rc * 128:(rc + 1) * 128, :], fo[:])
      except _Stop:
          pass
    nc.compile()
    return nc


def kernel(V, K, Q, padding_mask, Wv_w, Wv_b, Wk_w, Wk_b, Wq_w, Wq_b,
           Wo_w, Wo_b):
    from concourse.bass_utils import run_bass_kernel_spmd
    import ml_dtypes

    bf = ml_dtypes.bfloat16
    V = np.asarray(V, np.float32)
    K = np.asarray(K, np.float32)
    Q = np.asarray(Q, np.float32)
    padding_mask = np.asarray(padding_mask)

    if "nc" not in _BUILT:
        _BUILT["nc"] = _build_nc()
    nc = _BUILT["nc"]

    wk_t = np.ascontiguousarray(np.asarray(Wk_w, np.float32).T).astype(bf)
    wv_t = np.ascontiguousarray(np.asarray(Wv_w, np.float32).T).astype(bf)
    wq_t = np.ascontiguousarray(np.asarray(Wq_w, np.float32).T).astype(bf)
    wo_t = np.ascontiguousarray(np.asarray(Wo_w, np.float32).T).astype(bf)
    bk_s = np.ascontiguousarray(
        np.asarray(Wk_b, np.float32).reshape(8, 128).T)
    bq_s = np.ascontiguousarray(
        np.asarray(Wq_b, np.float32).reshape(8, 128).T)
    bv_rep = np.broadcast_to(
        np.asarray(Wv_b, np.float32).reshape(1, D), (128, D)).copy()
    bo_rep = np.broadcast_to(
        np.asarray(Wo_b, np.float32).reshape(1, D), (128, D)).copy()
    onesf = np.ones((128, 128), np.float32)

    xk_T = [np.ascontiguousarray(K[b].T).astype(bf) for b in range(B)]
    xv_T = [np.ascontiguousarray(V[b].T).astype(bf) for b in range(B)]

    in_maps = []
    blocks = []
    for core in range(NCORES):
        b, g = core // 4, core % 4
        stripes = SLOT_STRIPES[g]
        blocks.append((b, stripes))
        rows = np.concatenate(
            [np.arange(128 * st, 128 * st + 128) for st in stripes])
        xq_t = np.ascontiguousarray(Q[b][rows].T).astype(bf)
        maskin = np.zeros((128, 4 * NQ), np.float32)
        karange = np.arange(128)
        for q in range(4):
            qidx = 128 * stripes[q] + karange
            for t in range(4):
                kb = 4 * q + t
                kidx = 128 * kb + karange
                valid = (kidx[:, None] <= qidx[None, :]) & \
                        (padding_mask[b][kidx][:, None] != 0)
                maskin[:, 512 * q + 128 * t:512 * q + 128 * t + 128] = \
                    np.where(valid, 1.0, 0.0)
        in_maps.append({
            "xk_t": xk_T[b], "xv_t": xv_T[b], "xq_t": xq_t,
            "wk_t": wk_t, "wv_t": wv_t, "wq_t": wq_t, "wo_t": wo_t,
            "bk_s": bk_s, "bq_s": bq_s,
            "bv_rep": bv_rep, "bo_rep": bo_rep,
            "onesf": onesf,
            "maskin": maskin.astype(bf),
        })

    _BUILT["last_maps"] = in_maps
    res = run_bass_kernel_spmd(nc, in_maps, core_ids=list(range(NCORES)))
    _BUILT["last_result"] = res

    outf = np.empty((B, S, D), np.float32)
    for core in range(NCORES):
        b, stripes = blocks[core]
        o = res.results[core]["out"]
        for s, st in enumerate(stripes):
            outf[b, 128 * st:128 * st + 128] = o[128 * s:128 * s + 128]
    return outf


# revision 21
# speedup vs baseline: 1.3877x; 1.0823x over previous
"""Multi-headed causal attention (B=2, S=2048, D=1024, H=16, DK=DV=64) on 8
Trainium2 NeuronCores.

Sharding (zero-communication): cores split into 2 groups of 4, one group per
batch element. Within a group, core g owns four 128-query stripes, one per
"slot" s=0..3 with a key-block budget of 4(s+1) 128-key blocks. Stripe
assignment per group is chosen so every stripe's causal key-need fits its
slot budget:
    g0: stripes [0, 7, 8, 15], g1: [1, 6, 9, 14],
    g2: [2, 5, 10, 13],        g3: [3, 4, 11, 12]
The program is identical on all cores (SPMD); per-core variation lives only
in the data (query permutation in xq_t, 0/1 mask tiles, output row
unpermutation on host).

All matmul operands are bf16 (fast weight load, full PE rate at any free
size); accumulation is fp32 in PSUM. Scores for key-block kb are computed
only for query slots s >= kb//4 ("staircase"): lower slots never attend
those keys. exp(0.125*x) runs on the ACT engine per PSUM bank into bf16 am
tiles; masking is a multiplicative 0/1 bf16 op on am and touches only the
staircase edge (within quad q = kb//4, only slot-q's 128 columns can be
partial or dead). The softmax denominator comes from an all-ones 65th
column appended to each V tile; per-head normalization uses a fast DVE
reciprocal + PE row-replication on the small [64, 512] attention output,
letting the output projection accumulate all 16 heads in PSUM. Elementwise
engines only ever touch PSUM through single-bank [128, 512] fp32 tiles
(hardware constraint); projection bias-adds run on DVE/GpSimd
(tensor_scalar_add) to keep ACT free for exp.
"""

import numpy as np

B, S, D, H, DK = 2, 2048, 1024, 16, 64
NQ = 512          # queries per core: 4 slots x 128
NCORES = 8

SLOT_STRIPES = [
    [0, 7, 8, 15],
    [1, 6, 9, 14],
    [2, 5, 10, 13],
    [3, 4, 11, 12],
]

_BUILT = {}


def _build_nc():
    import os
    PH = int(os.environ.get("BISECT_PHASES", "9"))
    import concourse.bacc as bacc
    import concourse.mybir as mybir
    from concourse import tile

    f32 = mybir.dt.float32
    f32r = mybir.dt.float32r
    bf16 = mybir.dt.bfloat16
    AF = mybir.ActivationFunctionType
    ALU = mybir.AluOpType

    nc = bacc.Bacc("TRN2", target_bir_lowering=False, debug=False,
                   num_devices=NCORES)

    xk_t = nc.declare_dram_parameter("xk_t", [D, S], bf16, isOutput=False)
    xv_t = nc.declare_dram_parameter("xv_t", [D, S], bf16, isOutput=False)
    xq_t = nc.declare_dram_parameter("xq_t", [D, NQ], bf16, isOutput=False)
    wk_t = nc.declare_dram_parameter("wk_t", [D, D], bf16, isOutput=False)
    wv_t = nc.declare_dram_parameter("wv_t", [D, D], bf16, isOutput=False)
    wq_t = nc.declare_dram_parameter("wq_t", [D, D], bf16, isOutput=False)
    wo_t = nc.declare_dram_parameter("wo_t", [D, D], bf16, isOutput=False)
    bk_s = nc.declare_dram_parameter("bk_s", [128, 8], f32, isOutput=False)
    bq_s = nc.declare_dram_parameter("bq_s", [128, 8], f32, isOutput=False)
    bv_rep_d = nc.declare_dram_parameter("bv_rep", [128, D], f32,
                                         isOutput=False)
    bo_rep_d = nc.declare_dram_parameter("bo_rep", [128, D], f32,
                                         isOutput=False)
    onesf = nc.declare_dram_parameter("onesf", [128, 128], f32r,
                                  isOutput=False)
    maskin = nc.declare_dram_parameter("maskin", [128, 4 * NQ], bf16,
                                       isOutput=False)
    out = nc.declare_dram_parameter("out", [NQ, D], f32, isOutput=True)

    from contextlib import ExitStack

    class _Stop(Exception):
        pass

    with tile.TileContext(nc) as tc:
      try:
        with ExitStack() as ctx:
            persist = ctx.enter_context(tc.tile_pool(name="persist", bufs=1))

            # ---- constants / persistent tiles ----
            bk_sb = persist.tile([128, 8], f32, name="bk", tag="bk")
            bq_sb = persist.tile([128, 8], f32, name="bq", tag="bq")
            onesf_sb = persist.tile([128, 128], f32r, name="onesf", tag="onesf")
            mask_sb = persist.tile([128, 4 * NQ], bf16, name="mask",
                                   tag="mask")
            bv_rep = persist.tile([128, D], f32, name="bvrep", tag="bvrep")
            bo_rep = persist.tile([128, D], f32, name="borep", tag="borep")
            kT = [persist.tile([128, S], bf16, name=f"kt{ft}", tag=f"kt{ft}")
                  for ft in range(8)]
            qTz = [persist.tile([128, NQ], bf16, name=f"qt{h}",
                                tag=f"qt{h}")
                   for h in range(H)]
            # 66*H cols + 64 pad so the M=128 AV stationary stays in-bounds
            # for the last head
            v = [persist.tile([128, 66 * H + 64], bf16, name=f"v{kb}",
                              tag=f"v{kb}")
                 for kb in range(16)]
            navTn = [persist.tile([128, NQ], bf16, name=f"nv{i}",
                                  tag=f"nv{i}")
                     for i in range(8)]

            # p1pool (kT inputs) opened before p2pool so pool release is
            # LIFO (p2 closes first); DMA issue order below still puts the
            # q-phase inputs first.
            p1pool = tc.tile_pool(name="p1pool", bufs=1)
            p1s = p1pool.__enter__()
            p2pool = tc.tile_pool(name="p2pool", bufs=1)
            p2s = p2pool.__enter__()
            wq_sb = [p2s.tile([128, D], bf16, name=f"wq{kp}", tag=f"wq{kp}")
                     for kp in range(8)]
            xq_sb = [p2s.tile([128, NQ], bf16, name=f"xq{kp}", tag=f"xq{kp}")
                     for kp in range(8)]
            for kp in range(8):
                nc.sync.dma_start(wq_sb[kp][:],
                                  wq_t[kp * 128:(kp + 1) * 128, :])
                nc.sync.dma_start(xq_sb[kp][:],
                                  xq_t[kp * 128:(kp + 1) * 128, :])
            nc.sync.dma_start(bq_sb[:], bq_s[:])
            nc.sync.dma_start(bk_sb[:], bk_s[:])
            nc.sync.dma_start(onesf_sb[:], onesf[:])
            nc.sync.dma_start(mask_sb[:], maskin[:])
            nc.sync.dma_start(bv_rep[:], bv_rep_d[:])
            nc.sync.dma_start(bo_rep[:], bo_rep_d[:])

            wk_sb = [p1s.tile([128, D], bf16, name=f"wk{kp}", tag=f"wk{kp}")
                     for kp in range(8)]
            xk_sb = [p1s.tile([128, S], bf16, name=f"xk{kp}", tag=f"xk{kp}")
                     for kp in range(8)]
            for kp in range(8):
                nc.sync.dma_start(wk_sb[kp][:],
                                  wk_t[kp * 128:(kp + 1) * 128, :])
            for kp in range(8):
                nc.sync.dma_start(xk_sb[kp][:],
                                  xk_t[kp * 128:(kp + 1) * 128, :])

            # ---- P2: qT projection (first: smallest DMA footprint) ----
            if PH < 2:
                raise _Stop()
            for h in range(H):
                nc.gpsimd.memset(qTz[h][:], 0.0)
            with tc.tile_pool(name="psq", bufs=2, space="PSUM") as psq:
                for ft in range(8):
                    ps = psq.tile([128, NQ], f32, name="pq", tag="pq")
                    for kp in range(8):
                        nc.tensor.matmul(
                            ps[:], wq_sb[kp][:, ft * 128:(ft + 1) * 128],
                            xq_sb[kp][:], start=(kp == 0), stop=(kp == 7))
                    for i in range(2):
                        nc.vector.tensor_scalar_add(
                            qTz[2 * ft + i][64 * i:64 * i + 64, :],
                            ps[64 * i:64 * i + 64, :],
                            bq_sb[64 * i:64 * i + 64, ft:ft + 1])
            p2pool.__exit__(None, None, None)

            # ---- P1: kT projection ----
            if PH < 3:
                raise _Stop()
            with tc.tile_pool(name="psk", bufs=2, space="PSUM") as psk:
                for ft in range(8):
                    ps4 = [psk.tile([128, 512], f32, name=f"pk{sc}",
                                    tag=f"pk{sc}")
                           for sc in range(4)]
                    for kp in range(8):
                        for sc in range(4):
                            nc.tensor.matmul(
                                ps4[sc][:],
                                wk_sb[kp][:, ft * 128:(ft + 1) * 128],
                                xk_sb[kp][:, 512 * sc:512 * (sc + 1)],
                                start=(kp == 0), stop=(kp == 7))
                    for sc in range(4):
                        nc.vector.tensor_scalar_add(
                            kT[ft][:, 512 * sc:512 * (sc + 1)],
                            ps4[sc][:], bk_sb[:, ft:ft + 1])
            p1pool.__exit__(None, None, None)

            # ---- P3: V projection (single pass over xv, all 16 heads) ----
            if PH < 4:
                raise _Stop()
            # ones column (col 64 of each head's 66-wide group) for the
            # softmax denominator; pad col 65 is don't-care.
            for kb in range(16):
                nc.gpsimd.memset(
                    v[kb][:, 0:66 * H].rearrange("p (h c) -> p h c",
                                                 c=66)[:, :, 64:65],
                    1.0)
                nc.gpsimd.memset(v[kb][:, 66 * H:], 0.0)
            with tc.tile_pool(name="wvp", bufs=1) as wvp, \
                 tc.tile_pool(name="xvp", bufs=2) as xvp, \
                 tc.tile_pool(name="psv", bufs=3, space="PSUM") as psv:
                wv_sb = [wvp.tile([128, D], bf16, name=f"wv{kp}",
                                  tag=f"wv{kp}")
                         for kp in range(8)]
                for kp in range(8):
                    nc.sync.dma_start(wv_sb[kp][:],
                                      wv_t[kp * 128:(kp + 1) * 128, :])
                for chunk in range(4):
                    xvc = [xvp.tile([128, 512], bf16, name=f"xv{kp}",
                                    tag=f"xv{kp}")
                           for kp in range(8)]
                    for kp in range(8):
                        nc.sync.dma_start(
                            xvc[kp][:],
                            xv_t[kp * 128:(kp + 1) * 128,
                                 chunk * 512:(chunk + 1) * 512])
                    for stl in range(4):
                        kb = 4 * chunk + stl
                        pv = [psv.tile([128, 512], f32, name=f"pv{half}",
                                       tag=f"pv{half}")
                              for half in range(2)]
                        for kp in range(8):
                            for half in range(2):
                                nc.tensor.matmul(
                                    pv[half][:],
                                    xvc[kp][:, stl * 128:(stl + 1) * 128],
                                    wv_sb[kp][:, 512 * half:512 * (half + 1)],
                                    start=(kp == 0), stop=(kp == 7))
                        for half in range(2):
                            nc.vector.tensor_tensor(
                                v[kb][:, 528 * half:528 * (half + 1)]
                                .rearrange("p (h c) -> p h c", c=66)[
                                    :, :, 0:64],
                                pv[half][:].rearrange("p (h c) -> p h c",
                                                      c=64),
                                bv_rep[:, 512 * half:512 * (half + 1)]
                                .rearrange("p (h c) -> p h c", c=64),
                                ALU.add)

            # ---- P4: attention, software-pipelined over (head, kb-pair) ----
            if PH < 5:
                raise _Stop()
            with tc.tile_pool(name="scp", bufs=3, space="PSUM") as scp, \
                 tc.tile_pool(name="avp", bufs=3, space="PSUM") as avpp, \
                 tc.tile_pool(name="repp", bufs=2, space="PSUM") as repp, \
                 tc.tile_pool(name="dnmp", bufs=2) as dnmp, \
                 tc.tile_pool(name="amp", bufs=6) as amp:
                units = [(h, pr) for h in range(H) for pr in range(8)]
                avt = {}    # h -> avp psum tile
                amt = {}    # (h, pr) -> am tile
                dnms = {}   # h -> denominator sbuf tile
                deferred = []   # list of (due_idx, fn)

                def do_scores(h, pr):
                    hp = h // 2
                    q = pr // 2
                    Nq = 512 - 128 * q
                    qoff = 128 * q
                    am = amp.tile([128, 1024], bf16, name="am", tag="am")
                    for u in range(2):
                        kb = 2 * pr + u
                        sc = scp.tile([128, 512], f32, name="sc", tag="sc")
                        nc.tensor.matmul(
                            sc[:, 0:Nq],
                            kT[hp][:, kb * 128:(kb + 1) * 128],
                            qTz[h][:, qoff:512],
                            start=True, stop=True)
                        nc.scalar.activation(
                            am[:, 512 * u:512 * u + Nq], sc[:, 0:Nq],
                            AF.Exp, scale=0.125)
                    # multiplicative 0/1 mask on the staircase edge: only
                    # slot-q's 128 columns of each block can be partial/dead
                    amv = am[:].rearrange("p (u c) -> p u c",
                                          c=512)[:, :, 0:128]
                    moff = 256 * pr
                    mkv = mask_sb[:, moff:moff + 256].rearrange(
                        "p (u c) -> p u c", c=128)
                    eng = nc.vector if h % 2 == 0 else nc.gpsimd
                    eng.tensor_tensor(amv, amv, mkv, ALU.mult)
                    amt[(h, pr)] = am

                def do_av(h, pr):
                    q = pr // 2
                    Nq = 512 - 128 * q
                    qoff = 128 * q
                    am = amt.pop((h, pr))
                    for u in range(2):
                        kb = 2 * pr + u
                        nc.tensor.matmul(
                            avt[h][:, qoff:qoff + Nq],
                            v[kb][:, 66 * h:66 * h + 128],
                            am[:, 512 * u:512 * u + Nq],
                            start=(kb == 0), stop=(kb == 15))

                def do_norm_copy(h):
                    # denoms of a head pair: PSUM partition 64 -> SBUF rows
                    # 0 (even head) and 64 (odd head)
                    hp = h // 2
                    if h % 2 == 0:
                        dnm = dnmp.tile([128, NQ], f32, name="dnm",
                                        tag="dnm")
                        dnms[hp] = dnm
                    nc.scalar.copy(dnms[hp][64 * (h % 2):64 * (h % 2) + 1, :],
                                   avt[h][64:65, :])

                def make_norm_rep(hp):
                    def fn():
                        rcp = dnmp.tile([128, NQ], f32r, name="rcp",
                                        tag="rcp")
                        # rows other than 0/64 are junk and never read
                        with nc.allow_low_precision(reason="f32r recip"):
                            nc.vector.reciprocal(rcp[:], dnms.pop(hp)[:])
                        for i in range(2):
                            rep = repp.tile([128, NQ], f32, name="rep",
                                            tag="rep")
                            nc.tensor.matmul(
                                rep[0:64, :],
                                onesf_sb[64 * i:64 * i + 1, 0:64],
                                rcp[64 * i:64 * i + 1, :],
                                start=True, stop=True)
                            repS = dnmp.tile([128, NQ], f32, name="repS",
                                             tag="repS")
                            nc.vector.tensor_copy(repS[0:64, :],
                                                  rep[0:64, :])
                            with nc.allow_low_precision(
                                    reason="softmax norm"):
                                nc.vector.tensor_tensor(
                                    navTn[hp][64 * i:64 * i + 64, :],
                                    avt.pop(2 * hp + i)[0:64, :],
                                    repS[0:64, :], ALU.mult)
                    return fn

                for i, (h, pr) in enumerate(units):
                    if pr == 0:
                        avt[h] = avpp.tile([128, NQ], f32, name="av",
                                           tag="av")
                    do_scores(h, pr)
                    # run deferred items that are due
                    still = []
                    for due, fn in deferred:
                        if i >= due:
                            fn()
                        else:
                            still.append((due, fn))
                    deferred = still
                    if i >= 1:
                        ph, ppr = units[i - 1]
                        do_av(ph, ppr)
                        if ppr == 7:
                            do_norm_copy(ph)
                            if ph % 2 == 1:
                                deferred.append((i + 1,
                                                 make_norm_rep(ph // 2)))
                # drain
                ph, ppr = units[-1]
                do_av(ph, ppr)
                do_norm_copy(ph)
                for _, fn in deferred:
                    fn()
                make_norm_rep(ph // 2)()

            # ---- P5: output projection, all heads PSUM-accumulated ----
            if PH < 6:
                raise _Stop()
            with tc.tile_pool(name="wop", bufs=1) as wop, \
                 tc.tile_pool(name="fop", bufs=2) as fop, \
                 tc.tile_pool(name="pso", bufs=2, space="PSUM") as pso:
                wo_sb = [wop.tile([128, D], bf16, name=f"wo{hp}",
                                  tag=f"wo{hp}")
                         for hp in range(8)]
                for hp in range(8):
                    nc.sync.dma_start(wo_sb[hp][:],
                                      wo_t[hp * 128:(hp + 1) * 128, :])
                for rc in range(4):
                    po = [pso.tile([128, 512], f32, name=f"po{oc}",
                                   tag=f"po{oc}")
                          for oc in range(2)]
                    for hp in range(8):
                        for oc in range(2):
                            nc.tensor.matmul(
                                po[oc][:],
                                navTn[hp][:, rc * 128:(rc + 1) * 128],
                                wo_sb[hp][:, 512 * oc:512 * (oc + 1)],
                                start=(hp == 0), stop=(hp == 7))
                    fo = fop.tile([128, 1024], f32, name="fo", tag="fo")
                    for oc in range(2):
                        nc.vector.tensor_tensor(
                            fo[:, 512 * oc:512 * (oc + 1)],
                            po[oc][:],
                            bo_rep[:, 512 * oc:512 * (oc + 1)], ALU.add)
                    nc.sync.dma_start(out[rc * 128:(rc + 1) * 128, :], fo[:])
      except _Stop:
          pass
    nc.compile()
    return nc


def kernel(V, K, Q, padding_mask, Wv_w, Wv_b, Wk_w, Wk_b, Wq_w, Wq_b,
           Wo_w, Wo_b):
    from concourse.bass_utils import run_bass_kernel_spmd
    import ml_dtypes

    bf = ml_dtypes.bfloat16
    V = np.asarray(V, np.float32)
    K = np.asarray(K, np.float32)
    Q = np.asarray(Q, np.float32)
    padding_mask = np.asarray(padding_mask)

    if "nc" not in _BUILT:
        _BUILT["nc"] = _build_nc()
    nc = _BUILT["nc"]

    wk_t = np.ascontiguousarray(np.asarray(Wk_w, np.float32).T).astype(bf)
    wv_t = np.ascontiguousarray(np.asarray(Wv_w, np.float32).T).astype(bf)
    wq_t = np.ascontiguousarray(np.asarray(Wq_w, np.float32).T).astype(bf)
    wo_t = np.ascontiguousarray(np.asarray(Wo_w, np.float32).T).astype(bf)
    bk_s = np.ascontiguousarray(
        np.asarray(Wk_b, np.float32).reshape(8, 128).T)
    bq_s = np.ascontiguousarray(
        np.asarray(Wq_b, np.float32).reshape(8, 128).T)
    bv_rep = np.broadcast_to(
        np.asarray(Wv_b, np.float32).reshape(1, D), (128, D)).copy()
    bo_rep = np.broadcast_to(
        np.asarray(Wo_b, np.float32).reshape(1, D), (128, D)).copy()
    onesf = np.ones((128, 128), np.float32)

    xk_T = [np.ascontiguousarray(K[b].T).astype(bf) for b in range(B)]
    xv_T = [np.ascontiguousarray(V[b].T).astype(bf) for b in range(B)]

    in_maps = []
    blocks = []
    for core in range(NCORES):
        b, g = core // 4, core % 4
        stripes = SLOT_STRIPES[g]
        blocks.append((b, stripes))
        rows = np.concatenate(
            [np.arange(128 * st, 128 * st + 128) for st in stripes])
        xq_t = np.ascontiguousarray(Q[b][rows].T).astype(bf)
        maskin = np.zeros((128, 4 * NQ), np.float32)
        karange = np.arange(128)
        for q in range(4):
            qidx = 128 * stripes[q] + karange
            for t in range(4):
                kb = 4 * q + t
                kidx = 128 * kb + karange
                valid = (kidx[:, None] <= qidx[None, :]) & \
                        (padding_mask[b][kidx][:, None] != 0)
                maskin[:, 512 * q + 128 * t:512 * q + 128 * t + 128] = \
                    np.where(valid, 1.0, 0.0)
        in_maps.append({
            "xk_t": xk_T[b], "xv_t": xv_T[b], "xq_t": xq_t,
            "wk_t": wk_t, "wv_t": wv_t, "wq_t": wq_t, "wo_t": wo_t,
            "bk_s": bk_s, "bq_s": bq_s,
            "bv_rep": bv_rep, "bo_rep": bo_rep,
            "onesf": onesf,
            "maskin": maskin.astype(bf),
        })

    _BUILT["last_maps"] = in_maps
    res = run_bass_kernel_spmd(nc, in_maps, core_ids=list(range(NCORES)))
    _BUILT["last_result"] = res

    outf = np.empty((B, S, D), np.float32)
    for core in range(NCORES):
        b, stripes = blocks[core]
        o = res.results[core]["out"]
        for s, st in enumerate(stripes):
            outf[b, 128 * st:128 * st + 128] = o[128 * s:128 * s + 128]
    return outf
